# revision 16
# baseline (speedup 1.0000x reference)
"""Trainium2 Bass kernel for nn_Centerdist (segment variance loss).

Math: for each id k in [0, 1000):
    loss_k = sum_{i: id_i=k} ||x_i - mean_k||^2 / n_k
           = (sumsq_k - ||sums_k||^2 / n_k) / n_k
    loss = sum_k loss_k / n_uniq

Sharding strategy: rows are partitioned by id range — core c owns ids
[125c, 125(c+1)).  That makes each core's id window at most 125 wide, so
per-id aggregation needs a single 128-wide one-hot matmul per tile pair
(instead of eight 128-wide chunks covering the whole [0,1024) range when
rows are dealt round-robin).  Per 128-row tile the device:

  - squares x into the second half of the rhs tile (split between ACT
    and DVE to balance engine load),
  - builds the one-hot [128 rows, 128 window-ids] via is_equal against a
    window-relative iota (DVE),
  - a DoubleRow fp8 matmul accumulates one_hot.T @ [x | x^2] for TWO
    128-row tiles at once (256-deep contraction) into a persistent
    [128, 512] PSUM bank across all tiles.

The [window, 256] per-(id,d) sums and per-(id,d) sums-of-squares come
back per core; the host reduces the squares over d, combines the eight
disjoint windows, and applies the final per-id division and mean.
Inputs are pre-cast to fp8e4m3 on the host (quantization noise averages
out across ~262 rows/id and lands ~1e-3 relative on the loss) and laid
out so each SBUF partition's slice of a DMA load group is contiguous.
"""

import numpy as np

from concourse import bacc, bass, bass_utils, mybir, tile

F32 = mybir.dt.float32
F16 = mybir.dt.float16
F8 = mybir.dt.float8e4
F8NP = mybir.dt.np(F8)

N_FULL = 262144
D = 256
NUM_IDS = 1000
P = 128
N_CORES = 8
IDS_PER_CORE = (NUM_IDS + N_CORES - 1) // N_CORES  # 125
RW = 2 * D  # rhs width: [x | x^2]
LOAD_T = 16  # tiles per DMA load


def build_program(tiles: int, reps: int = 1):
    """Build the per-core Bass program processing `tiles` 128-row tiles.

    reps>1 repeats the whole pass (for slope-based HW timing); the output
    is identical since each rep restarts the PSUM accumulation group.
    """
    nc = bacc.Bacc(
        "TRN2",
        target_bir_lowering=False,
        debug=False,
        num_devices=N_CORES,
    )
    load_t = min(LOAD_T, tiles)
    assert tiles % load_t == 0 and load_t % 2 == 0
    n_groups = tiles // load_t
    # host pre-arranges x so each partition's group slice is contiguous:
    # x_d[g*P + p, t*D + d] = row (g*load_t + t)*P + p, feature d
    x_d = nc.dram_tensor("x", [n_groups * P, load_t * D], F8, kind="ExternalInput")
    idst_d = nc.dram_tensor("idst", [P, tiles], F32, kind="ExternalInput")
    iota_d = nc.dram_tensor("iota", [P, P], F16, kind="ExternalInput")
    out_d = nc.dram_tensor("out", [P, RW], F32, kind="ExternalOutput")

    with tile.TileContext(nc) as tc:
        with (
            tc.tile_pool(name="const", bufs=1) as cpool,
            tc.tile_pool(name="xp", bufs=4) as xpool,
            tc.tile_pool(name="sqp", bufs=3) as sqpool,
            tc.tile_pool(name="ohp", bufs=4) as ohpool,
            tc.tile_pool(name="psp", bufs=1, space="PSUM") as pspool,
            tc.tile_pool(name="evp", bufs=1) as evpool,
        ):
            # constants go on the ACT HWDGE queue so the first big x load
            # (SP queue) isn't stuck behind them
            iota_t = cpool.tile([P, P], F16, tag="iota")
            nc.scalar.dma_start(iota_t[:], iota_d.ap())
            idst_t = cpool.tile([P, tiles], F32, tag="idst")
            nc.scalar.dma_start(idst_t[:], idst_d.ap())

            psum_x = pspool.tile([P, D], F32, name="psx", tag="psx")
            psum_q = pspool.tile([P, D], F32, name="psq", tag="psq")

            x_g = x_d.ap().rearrange("(g p) (t d) -> g p t d", p=P, t=load_t)

            for rep in range(reps):
                for gi in range(n_groups):
                    xt = xpool.tile([P, load_t, D], F8, name="xt", tag="xt")
                    if gi == 0 and load_t > 2:
                        # split the first load so the pipeline starts filling
                        # after a couple of tiles instead of a full group
                        nc.sync.dma_start(xt[:, 0:2, :], x_g[gi, :, 0:2])
                        nc.sync.dma_start(xt[:, 2:load_t, :], x_g[gi, :, 2:load_t])
                    else:
                        nc.sync.dma_start(xt[:, :, :], x_g[gi])

                    # x^2 in fp16 (fp8 squares carry a ~-0.8% systematic
                    # quantization bias on this distribution).  GPSIMD
                    # squares the first half tile-by-tile (so the group's
                    # first matmuls start right after the DMA lands) and ACT
                    # squares the second half in two batched ops; DVE is left
                    # free for the one-hots.
                    sq = sqpool.tile([P, load_t, D], F16, name="sq", tag="sq")
                    n_pool = load_t // 2
                    for tt in range(n_pool):
                        nc.gpsimd.tensor_tensor(
                            out=sq[:, tt, :],
                            in0=xt[:, tt, :],
                            in1=xt[:, tt, :],
                            op=mybir.AluOpType.mult,
                        )
                    if n_pool < load_t:
                        nc.scalar.activation(
                            sq[:, n_pool:load_t, :],
                            xt[:, n_pool:load_t, :],
                            mybir.ActivationFunctionType.Square,
                        )

                    for tt in range(0, load_t, 2):
                        t = gi * load_t + tt
                        # one-hots for the tile pair, [Ki, Ko=2, window]
                        oh = ohpool.tile([P, 2, P], F8, name="oh", tag="oh")
                        for k in range(2):
                            nc.vector.tensor_scalar(
                                out=oh[:, k, :],
                                in0=iota_t[:],
                                scalar1=idst_t[:, t + k : t + k + 1],
                                scalar2=None,
                                op0=mybir.AluOpType.is_equal,
                            )

                        # per-id x sums: one DoubleRow fp8 matmul per pair
                        nc.tensor.matmul(
                            psum_x[:],
                            oh[:, :, :],
                            xt[:, tt : tt + 2, :],
                            start=(t == 0),
                            stop=(t == tiles - 2),
                            perf_mode=mybir.MatmulPerfMode.DoubleRow,
                        )
                        # per-(id,d) sums of squares: fp16 rhs, same fp8
                        # one-hot as stationary
                        for k in range(2):
                            nc.tensor.matmul(
                                psum_q[:],
                                oh[:, k, :],
                                sq[:, tt + k, :],
                                start=(t + k == 0),
                                stop=(t + k == tiles - 1),
                            )

            ev = evpool.tile([P, RW], F32, name="ev", tag="ev")
            nc.vector.tensor_copy(ev[:, 0:D], psum_x[:])
            nc.vector.tensor_copy(ev[:, D:RW], psum_q[:])
            nc.sync.dma_start(out_d.ap(), ev[:])

    nc.compile()
    return nc


_PROGRAM_CACHE: dict = {}


def _get_program(tiles: int, reps: int = 1):
    key = (tiles, reps)
    if key not in _PROGRAM_CACHE:
        _PROGRAM_CACHE[key] = build_program(tiles, reps)
    return _PROGRAM_CACHE[key]


def make_in_maps(reid_feat: np.ndarray, ids: np.ndarray):
    """Partition rows by id range across the 8 cores.

    Core c gets all rows whose id is in [125c, 125(c+1)), padded with
    rel-id -1 rows (which match nothing) to a common tile count.
    """
    x = np.asarray(reid_feat, dtype=np.float32)
    ids_np = np.asarray(ids).astype(np.int64)
    valid = ids_np >= 0

    order = np.argsort(ids_np, kind="stable")
    ids_sorted = ids_np[order]
    # drop invalid (negative) ids — they contribute nothing to the sums
    lo_valid = np.searchsorted(ids_sorted, 0, side="left")
    bounds = np.searchsorted(
        ids_sorted, np.arange(0, NUM_IDS + IDS_PER_CORE, IDS_PER_CORE), side="left"
    )
    bounds[0] = lo_valid
    counts_per_core = np.diff(bounds)
    max_rows = int(counts_per_core.max())
    tiles = max(2, (max_rows + P - 1) // P)
    if tiles > LOAD_T:
        tiles = ((tiles + LOAD_T - 1) // LOAD_T) * LOAD_T  # whole DMA groups
    elif tiles % 2:
        tiles += 1
    ns = tiles * P
    load_t = min(LOAD_T, tiles)
    n_groups = tiles // load_t

    iota = np.broadcast_to(
        np.arange(P, dtype=np.float16), (P, P)
    ).copy()  # iota[p, j] = j

    in_maps = []
    for c in range(N_CORES):
        sel = order[bounds[c] : bounds[c + 1]]
        n_c = sel.shape[0]
        xs = np.zeros((ns, D), dtype=F8NP)
        xs[:n_c] = x[sel].astype(F8NP)
        # contiguous-per-partition layout: [g, p, t*D+d]
        xdev = (
            xs.reshape(n_groups, load_t, P, D)
            .transpose(0, 2, 1, 3)
            .reshape(n_groups * P, load_t * D)
            .copy()
        )
        rel = np.full((ns,), -1.0, dtype=np.float32)
        rel[:n_c] = (ids_sorted[bounds[c] : bounds[c + 1]] - IDS_PER_CORE * c).astype(
            np.float32
        )
        idst = rel.reshape(tiles, P).T.copy()
        in_maps.append({"x": xdev, "idst": idst, "iota": iota})
    return in_maps, tiles, valid


def finalize(parts: np.ndarray, ids: np.ndarray, valid: np.ndarray) -> np.ndarray:
    """Combine per-core window partials [cores, 128, 512] into the loss."""
    parts = parts.astype(np.float64)
    K_PAD = IDS_PER_CORE * N_CORES + P
    sums = np.zeros((K_PAD, D))
    sumsq = np.zeros((K_PAD,))
    for c in range(N_CORES):
        base = IDS_PER_CORE * c
        sums[base : base + P] += parts[c, :, 0:D]
        sumsq[base : base + P] += parts[c, :, D:RW].sum(axis=1)
    sums = sums[:NUM_IDS]
    sumsq = sumsq[:NUM_IDS]

    ids_np = np.asarray(ids).astype(np.int64)
    counts = np.bincount(ids_np[valid], minlength=NUM_IDS)[:NUM_IDS].astype(np.float64)
    safe_n = np.maximum(counts, 1.0)
    sq_per_id = sumsq - (sums * sums).sum(axis=1) / safe_n
    per_id_loss = np.where(counts > 0, sq_per_id / safe_n, 0.0)
    n_uniq = float((counts > 0).sum()) + (1.0 if (~valid).any() else 0.0)
    return np.array(per_id_loss.sum() / n_uniq, dtype=np.float32)


def run_device(reid_feat, ids, trace: bool = False):
    in_maps, tiles, valid = make_in_maps(reid_feat, ids)
    nc = _get_program(tiles)
    res = bass_utils.run_bass_kernel_spmd(
        nc, in_maps, core_ids=list(range(N_CORES)), trace=trace
    )
    parts = np.stack([res.results[c]["out"] for c in range(N_CORES)])
    return parts, valid, res


class DeviceRunner:
    """Persistent jitted SPMD executor (mirrors bass2jax.run_bass_via_pjrt)
    so a program can be executed many times for timing without re-tracing."""

    def __init__(self, nc, in_maps, chain: int = 1):
        import jax
        from jax.sharding import Mesh, PartitionSpec
        from jax.experimental.shard_map import shard_map
        from concourse import bass2jax, mybir as mb

        bass2jax.install_neuronx_cc_hook()
        partition_name = (
            nc.partition_id_tensor.name if nc.partition_id_tensor else None
        )
        in_names, out_names, out_avals, zero_outs = [], [], [], []
        for alloc in nc.m.functions[0].allocations:
            if not isinstance(alloc, mb.MemoryLocationSet):
                continue
            name = alloc.memorylocations[0].name
            if alloc.kind == "ExternalInput":
                if name != partition_name:
                    in_names.append(name)
            elif alloc.kind == "ExternalOutput":
                shape = tuple(alloc.tensor_shape)
                npdt = np.dtype(mb.dt.np(alloc.dtype))
                out_names.append(name)
                out_avals.append(jax.core.ShapedArray(shape, npdt))
                zero_outs.append(np.zeros(shape, npdt))
        self.out_names = out_names
        n_params = len(in_names)
        n_outs = len(out_avals)
        all_names = list(in_names) + list(out_names)
        if partition_name is not None:
            all_names.append(partition_name)

        def _body(*args):
            ins = list(args[:n_params])
            outs = list(args[n_params:])
            # chain>1 = several dependent NEFF executions per dispatch, so
            # per-dispatch overhead can be sloped away when timing
            for _ in range(chain):
                operands = ins + outs
                if partition_name is not None:
                    operands.append(bass2jax.partition_id_tensor())
                outs = list(
                    bass2jax._bass_exec_p.bind(
                        *operands,
                        out_avals=tuple(out_avals),
                        in_names=tuple(all_names),
                        out_names=tuple(out_names),
                        lowering_input_output_aliases=(),
                        sim_require_finite=True,
                        sim_require_nnan=True,
                        nc=nc,
                    )
                )
            return tuple(outs)

        devices = jax.devices()[:N_CORES]
        mesh = Mesh(np.asarray(devices), ("core",))
        in_specs = (PartitionSpec("core"),) * (n_params + n_outs)
        out_specs = (PartitionSpec("core"),) * n_outs
        self._fn = jax.jit(
            shard_map(
                _body,
                mesh=mesh,
                in_specs=in_specs,
                out_specs=out_specs,
                check_rep=False,
            ),
            keep_unused=True,
        )
        self._jax = jax
        concat_in = [
            np.concatenate([np.asarray(in_maps[c][nm]) for c in range(N_CORES)], axis=0)
            for nm in in_names
        ]
        concat_zeros = [
            np.zeros((N_CORES * z.shape[0], *z.shape[1:]), z.dtype) for z in zero_outs
        ]
        sharding = jax.sharding.NamedSharding(mesh, PartitionSpec("core"))
        self._args = [jax.device_put(a, sharding) for a in concat_in + concat_zeros]
        self.out_shapes = [a.shape for a in out_avals]

    def run_once(self):
        outs = self._fn(*self._args)
        self._jax.block_until_ready(outs)
        return outs

    def results(self):
        outs = self.run_once()
        return [
            {
                nm: np.asarray(outs[i]).reshape(N_CORES, *self.out_shapes[i])[c]
                for i, nm in enumerate(self.out_names)
            }
            for c in range(N_CORES)
        ]

    def time_exec(self, iters: int = 20, warmup: int = 3):
        import time as _time

        for _ in range(warmup):
            self.run_once()
        times = []
        for _ in range(iters):
            t0 = _time.perf_counter()
            self.run_once()
            times.append(_time.perf_counter() - t0)
        return float(np.median(times)), times


def kernel(reid_feat, ids) -> np.ndarray:
    parts, valid, _ = run_device(reid_feat, ids)
    return finalize(parts, np.asarray(ids), valid)
